# revision 67
# baseline (speedup 1.0000x reference)
"""MoE router gate kernel for Trainium2 (8 NeuronCores, SPMD data-parallel).

Reference computation (per problem nn_Gate_7241314861587):
    logits = x @ weight.T          # [8192, 4096] @ [4096, 256] -> [8192, 256]
    scores = sigmoid(logits)
    topv, indices = top_k(scores, 8)
    gates = topv / sum(topv)
    returns (gates f32 [8192, 8], indices int32 [8192, 8])

Strategy (fp16 main pass + fp8e4 DoubleRow correction passes, all
accumulating in one 2^26-scaled PSUM group per token tile):
  - Data parallel: 1024 tokens per core; router weight replicated.
  - x = xh + xl with xh = fp16(x) shipped pre-scaled by 2^13 and xl
    shipped as e4m3 fp8 scaled by 2^16 (3 bytes/element on the wire).
  - w = wh + wl with wh = fp16(w) shipped pre-scaled by 2^13 and wl
    shipped as e4m3 scaled by 2^21.
  - logits*2^26 = (xh*2^13)@(wh*2^13)      (fp16, 1 cycle/row)
               + (xh*2^5)8@(wl*2^21)8     (fp8e4 DoubleRow)
               + (xl*2^16)8@(wh*2^10)8    (fp8e4 DoubleRow)
    DoubleRow runs 0.5 cycles/row with a 256-deep contraction per
    instruction (4x cheaper per flop than fp16); the fp8 operands xh8 /
    wh8 are cast on-chip (DVE/ACT/Pool).  Every pass lands at the same
    2^26 scale, so all three accumulate into ONE PSUM bank per tile --
    no combine arithmetic at all.
  - Top-8 selection (DVE MAX8 / FIND_INDEX_8) runs directly on the
    scaled PSUM: max and argmax are invariant to positive scaling, and
    sigmoid is monotonic.  The host rescales the 8 winning logits by
    2^-26 and applies sigmoid + gate normalization (8192x8 numpy).
  - Logit error std ~1e-5: top-8 indices match the fp32 reference except
    a couple of near-ties (idx rel err ~2e-3, gates ~8e-7).
  - A grouped warm-up matmul chain at t=0 brings the PE out of the mid
    p-state before real data lands (the cost model ramps 1.2->2.4 GHz
    after ~3us of gapless PE activity).
  - Schedule: P23(t) is enqueued 4 tiles behind P1(t) ("d=4"), with the
    DMA stream ordered so every tensor's (arrival time + remaining
    PE-FIFO work) chain is roughly equal; the last xl8 tiles arrive in
    quarter chunks and gate only the cheap final DR passes.
"""

import numpy as np

TOKENS, DIM, N_EXPERTS, TOPK = 8192, 4096, 256, 8
N_CORES = 8
TOK_SHARD = TOKENS // N_CORES     # 1024
TT = TOK_SHARD // 128             # 8 token tiles per core
KC = DIM // 128                   # 32 contraction chunks

_compiled = None


def _build(variant="hybrid"):
    import concourse.mybir as mybir
    import concourse.tile as tile
    from concourse import bacc

    f32 = mybir.dt.float32
    f16 = mybir.dt.float16
    f8 = mybir.dt.float8e4
    u32 = mybir.dt.uint32
    DR = mybir.MatmulPerfMode.DoubleRow
    Copy = mybir.ActivationFunctionType.Copy

    nc = bacc.Bacc("TRN2", target_bir_lowering=False, debug=False)

    xh_d = nc.dram_tensor("xh", [TT, 128, KC * 128], f16, kind="ExternalInput")
    xl8_d = nc.dram_tensor("xl8", [TT, 128, KC * 128], f8, kind="ExternalInput")
    wh_d = nc.dram_tensor("wh", [128, KC * 256], f16, kind="ExternalInput")
    wl8_d = nc.dram_tensor("wl8", [128, KC * 256], f8, kind="ExternalInput")
    out_d = nc.dram_tensor("out", [TOK_SHARD, 2 * TOPK], u32, kind="ExternalOutput")

    wh_v = wh_d[:].rearrange("p (kc e) -> p kc e", kc=KC)
    wl8_v = wl8_d[:].rearrange("p (kc e) -> p kc e", kc=KC)

    with tile.TileContext(nc) as tc:
        with (
            tc.tile_pool(name="wp", bufs=1) as wp,
            tc.tile_pool(name="xp", bufs=8) as xp,
            tc.tile_pool(name="x8p", bufs=8) as x8p,
            tc.tile_pool(name="pp", bufs=3, space="PSUM") as pp,
            tc.tile_pool(name="sp", bufs=3) as sp,
            tc.tile_pool(name="prp", bufs=8) as prp,
            tc.tile_pool(name="op", bufs=1) as op,
        ):
            # --- PE warm-up: one grouped accumulation chain, no data deps.
            warm_in = wp.tile([128, 512], f16, tag="warm_in")
            warm_ps = pp.tile([128, 512], f32, tag="warm_ps", bufs=1)
            nc.vector.memset(warm_in[:], 0)
            WARM = 10
            for i in range(WARM):
                nc.tensor.matmul(
                    warm_ps[:], warm_in[:, 0:128], warm_in[:],
                    start=(i == 0), stop=(i == WARM - 1),
                    skip_group_check=True)

            wh_t = wp.tile([128, KC, 256], f16, tag="wh")
            wl8_t = wp.tile([128, KC, 256], f8, tag="wl8")
            wh8_t = wp.tile([128, KC, 256], f8, tag="wh8")
            out_stage = op.tile([128, TT, 2 * TOPK], u32, tag="outs")

            xh_t = [None] * TT
            xl8_t = [None] * TT
            xh8_t = [None] * TT
            for t in range(TT):
                xh_t[t] = xp.tile([128, KC, 128], f16, tag="xh", name=f"xh{t}")
                xl8_t[t] = xp.tile([128, KC, 128], f8, tag="xl8", name=f"xl8_{t}")
                xh8_t[t] = x8p.tile([128, KC, 128], f8, tag="xh8", name=f"xh8_{t}")

            WCH = 4

            def dma_wh(c):
                nc.sync.dma_start(
                    wh_t[:, c * WCH:(c + 1) * WCH, :],
                    wh_v[:, c * WCH:(c + 1) * WCH, :])

            def dma_xh(t, half):
                xh_v = xh_d[t].rearrange("p (kc n) -> p kc n", kc=KC)
                h = KC // 2
                sl = slice(half * h, (half + 1) * h)
                nc.sync.dma_start(xh_t[t][:, sl, :], xh_v[:, sl, :])

            def dma_xhq(t, q):
                xh_v = xh_d[t].rearrange("p (kc n) -> p kc n", kc=KC)
                h = KC // 4
                sl = slice(q * h, (q + 1) * h)
                nc.sync.dma_start(xh_t[t][:, sl, :], xh_v[:, sl, :])

            def dma_xl8h(t, half):
                xl_v = xl8_d[t].rearrange("p (kc n) -> p kc n", kc=KC)
                h = KC // 2
                sl = slice(half * h, (half + 1) * h)
                nc.sync.dma_start(xl8_t[t][:, sl, :], xl_v[:, sl, :])

            def dma_xl8q(t, q):
                xl_v = xl8_d[t].rearrange("p (kc n) -> p kc n", kc=KC)
                h = KC // 4
                sl = slice(q * h, (q + 1) * h)
                nc.sync.dma_start(xl8_t[t][:, sl, :], xl_v[:, sl, :])

            # d=4 interleave: five xh tiles stream before the fp8 side so
            # every arrival's (arrival + remaining-FIFO-work) chain is ~equal;
            # the last xl8 tiles land last, gating only cheap DR passes.
            dma_wh(0); dma_xhq(0, 0); dma_wh(1); dma_xhq(0, 1)
            dma_wh(2); dma_xhq(0, 2); dma_wh(3); dma_xhq(0, 3)
            dma_wh(4); dma_xhq(1, 0); dma_wh(5); dma_xhq(1, 1)
            dma_wh(6); dma_wh(7)
            dma_xhq(1, 2); dma_xhq(1, 3)
            dma_xhq(2, 0); dma_xhq(2, 1); dma_xhq(2, 2); dma_xhq(2, 3)
            dma_xhq(3, 0); dma_xhq(3, 1); dma_xhq(3, 2); dma_xhq(3, 3)
            dma_xhq(4, 0); dma_xhq(4, 1); dma_xhq(4, 2); dma_xhq(4, 3)
            dma_xhq(5, 0); dma_xhq(5, 1); dma_xhq(5, 2); dma_xhq(5, 3)
            nc.sync.dma_start(wl8_t[:, 0:KC // 2, :], wl8_v[:, 0:KC // 2, :])
            dma_xl8q(0, 0); dma_xl8q(0, 1); dma_xl8q(0, 2); dma_xl8q(0, 3)
            nc.sync.dma_start(wl8_t[:, KC // 2:KC, :], wl8_v[:, KC // 2:KC, :])
            dma_xl8q(1, 0); dma_xl8q(1, 1); dma_xl8q(1, 2); dma_xl8q(1, 3)
            dma_xhq(6, 0); dma_xhq(6, 1); dma_xhq(6, 2); dma_xhq(6, 3)
            dma_xl8q(2, 0); dma_xl8q(2, 1); dma_xl8q(2, 2); dma_xl8q(2, 3)
            dma_xhq(7, 0); dma_xhq(7, 1); dma_xhq(7, 2); dma_xhq(7, 3)
            dma_xl8q(3, 0); dma_xl8q(3, 1); dma_xl8q(3, 2); dma_xl8q(3, 3)
            dma_xl8h(4, 0); dma_xl8h(4, 1); dma_xl8h(5, 0); dma_xl8h(5, 1)
            dma_xl8q(6, 0); dma_xl8q(6, 1); dma_xl8q(6, 2); dma_xl8q(6, 3)
            dma_xl8q(7, 0); dma_xl8q(7, 1); dma_xl8q(7, 2); dma_xl8q(7, 3)

            # wh8 = fp8(wh * 2^10) on the scalar engine, chunked behind the
            # wh DMA chunks.
            for c in range(0, KC, 8):
                nc.scalar.activation(
                    wh8_t[:, c:c + 8, :], wh_t[:, c:c + 8, :], Copy,
                    scale=0.125)

            cast_eng = ["dve", "act", "pool", "dve", "act", "pool",
                        "dve", "act"]

            def cast_xh8(t):
                if cast_eng[t] == "act":
                    nc.scalar.activation(
                        xh8_t[t][:], xh_t[t][:], Copy, scale=2.0 ** -8)
                elif cast_eng[t] == "dve":
                    nc.vector.tensor_scalar_mul(
                        xh8_t[t][:], xh_t[t][:], 2.0 ** -8)
                else:
                    nc.gpsimd.tensor_scalar_mul(
                        xh8_t[t][:], xh_t[t][:], 2.0 ** -8)

            ps = [None] * TT

            def p1(t):
                ps[t] = pp.tile([128, 256], f32, tag="ps",
                                name=f"ps{t}", bufs=6)
                for k in range(KC):
                    nc.tensor.matmul(
                        ps[t][:], xh_t[t][:, k, :], wh_t[:, k, :],
                        start=(k == 0), stop=False,
                        skip_group_check=True)

            def p23(t):
                for k in range(0, KC, 2):
                    nc.tensor.matmul(
                        ps[t][:], xl8_t[t][:, k:k + 2, :],
                        wh8_t[:, k:k + 2, :],
                        start=False, stop=False, perf_mode=DR,
                        skip_group_check=True)
                for k in range(0, KC, 2):
                    nc.tensor.matmul(
                        ps[t][:], xh8_t[t][:, k:k + 2, :],
                        wl8_t[:, k:k + 2, :],
                        start=False, stop=(k == KC - 2), perf_mode=DR,
                        skip_group_check=True)

            def finish(t):
                # top-8 directly on the 2^26-scaled psum: max/max_index are
                # scale-invariant; the host rescales the 8 winners before
                # sigmoid.  gates-f32 and idx-u32 share one staging tile.
                gview = out_stage[:, t, 0:TOPK].bitcast(f32)
                nc.vector.max(out=gview, in_=ps[t][:])
                nc.vector.max_index(
                    out=out_stage[:, t, TOPK:2 * TOPK], in_max=gview,
                    in_values=ps[t][:])

            for t in range(5):
                cast_xh8(t)
                p1(t)
            cast_xh8(5)
            p1(5)
            p23(0)
            finish(0)
            p23(1)
            cast_xh8(6)
            p1(6)
            finish(1)
            p23(2)
            cast_xh8(7)
            p1(7)
            finish(2)
            for t in range(3, TT):
                p23(t)
                finish(t)

            out_v = out_d[:].rearrange("(t p) k -> p t k", t=TT)
            nc.sync.dma_start(out_v[:, 0:7, :], out_stage[:, 0:7, :])
            nc.sync.dma_start(out_v[:, 7:TT, :], out_stage[:, 7:TT, :])

    nc.compile()
    return nc


def _prep_inputs(x, weight):
    """Host-side shard + transpose + fp16/fp8 split -> per-core in_maps."""
    import ml_dtypes
    e4 = ml_dtypes.float8_e4m3

    x = np.ascontiguousarray(np.asarray(x, dtype=np.float32))
    w = np.ascontiguousarray(np.asarray(weight, dtype=np.float32))

    wT = np.ascontiguousarray(w.T)                     # [4096, 256]
    wh = wT.astype(np.float16)
    wl8 = ((wT - wh.astype(np.float32)) * np.float32(2.0 ** 21)).astype(e4)
    wh = wh * np.float16(2.0 ** 13)     # exact power-of-2 scale in fp16

    def pack_w(a):
        return np.ascontiguousarray(
            a.reshape(KC, 128, N_EXPERTS).transpose(1, 0, 2).reshape(
                128, KC * N_EXPERTS))

    wh_p = pack_w(wh)
    wl8_p = pack_w(wl8)

    xh = x.astype(np.float16)
    xl8 = ((x - xh.astype(np.float32)) * np.float32(2.0 ** 16)).astype(e4)
    xh = xh * np.float16(2.0 ** 13)     # exact power-of-2 scale in fp16

    def pack_x(a):
        # [1024, 4096] -> [t, tok, kc, p] -> [t, p, kc, tok]
        b = a.reshape(TT, 128, KC, 128).transpose(0, 3, 2, 1)
        return np.ascontiguousarray(b.reshape(TT, 128, KC * 128))

    in_maps = []
    for c in range(N_CORES):
        sl = slice(c * TOK_SHARD, (c + 1) * TOK_SHARD)
        in_maps.append({
            "xh": pack_x(xh[sl]),
            "xl8": pack_x(xl8[sl]),
            "wh": wh_p,
            "wl8": wl8_p,
        })
    return in_maps


def kernel(x, weight, _trace=False, _trace_kwargs=None):
    global _compiled
    from concourse.bass_utils import run_bass_kernel_spmd

    if _compiled is None:
        _compiled = _build()

    in_maps = _prep_inputs(x, weight)
    res = run_bass_kernel_spmd(
        _compiled,
        in_maps,
        core_ids=list(range(N_CORES)),
        trace=_trace,
        **(_trace_kwargs or {}),
    )

    out = np.concatenate([r["out"] for r in res.results], axis=0)
    top_logits = out[:, 0:TOPK].view(np.float32) * np.float32(2.0 ** -26)
    idx = out[:, TOPK:2 * TOPK].astype(np.int32)
    topv = 1.0 / (1.0 + np.exp(-top_logits))
    gates = topv / topv.sum(axis=-1, keepdims=True)
    if _trace:
        kernel.last_results = res
    return gates, idx
